# revision 7
# baseline (speedup 1.0000x reference)
"""Trainium2 Bass kernel for BaseFisheyeLSSTransform (BEV pooling), v3.

Architecture (8-core SPMD, one program, per-core data tables):
- Host (index-only math): voxelize the frustum geometry on jax-cpu, find
  runs of kept points (consecutive source rows, same output x-row), encode
  them as indirect-DMA descriptors in two classes (L=1 single rows, L=4
  spans). Slots (b, x-row) are balanced across cores; per-(slot, class)
  descriptor counts are quantized and maxed over cores so all 8 cores share
  one instruction structure.
- Device: x is staged as one concatenated [2*566400, 80] fp16 tensor. Per
  instruction one indirect DMA gathers 128 descriptors into SBUF
  [128, L*80] fp16. Per (instruction, lane, slot) segment, a single fused
  DVE op builds M = (iota == vid) * invcnt in fp16 ([128, 360]); rows
  outside the segment (padding, other slots) carry vid = -1 so M is zero
  there and the fp16 matmul can contract over all 128 partitions without
  masking: PSUM[slot] += g_lane^T @ M. Closed slots are copied
  PSUM -> slab on the Scalar engine and flushed to DRAM in 16-slot blocks.
- Host assembles [2, 80, 360, 360] from the 8 slabs (pure unshard).
"""
import sys

sys.path.insert(0, "/opt/trn_rl_repo")

import numpy as np

B, N, C = 2, 4, 80
FH, FW, D = 40, 60, 59
NX, NY = 360, 360
PB = N * D * FH * FW  # 566400 rows per batch
P = 128
CLASSES = (2,)
QUANT = {2: 32}
FLUSH_WINDOWS = 16


# ---------------------------------------------------------------- host side


def _geometry(camera2lidar_rots, camera2lidar_trans):
    import jax
    import jax.numpy as jnp

    cpu = jax.devices("cpu")[0]
    with jax.default_device(cpu):
        DX = jnp.array([0.3, 0.3, 8.0], dtype=jnp.float32)
        ORIGIN = jnp.array([-54.0, -54.0, -5.0], dtype=jnp.float32)
        ds = jnp.arange(1.0, 60.0, 1.0, dtype=jnp.float32)
        az = jnp.linspace(-1.92, 1.92, FW, dtype=jnp.float32)
        el = jnp.linspace(-0.61, 0.61, FH, dtype=jnp.float32)
        d_, e_, a_ = ds[:, None, None], el[None, :, None], az[None, None, :]
        xs = d_ * jnp.cos(e_) * jnp.sin(a_)
        ys = jnp.broadcast_to(d_ * jnp.sin(e_), (D, FH, FW))
        zs = d_ * jnp.cos(e_) * jnp.cos(a_)
        fr = jnp.stack([xs, ys, zs], axis=-1)
        geom = jnp.einsum("bnij,dhwj->bndhwi", camera2lidar_rots, fr)
        geom = geom + camera2lidar_trans[:, :, None, None, None, :]
        coords = np.asarray(((geom - ORIGIN) / DX).astype(jnp.int32))
    kept = (
        (coords[..., 0] >= 0) & (coords[..., 0] < NX)
        & (coords[..., 1] >= 0) & (coords[..., 1] < NY)
        & (coords[..., 2] >= 0) & (coords[..., 2] < 1)
    )
    return coords, kept


def _build_runs(coords, kept):
    """rows[(b, xrow)] = {L: [(global_start, ys[L], ws[L])]}; ys=-1 pads."""
    Lmax = max(CLASSES)
    rows = {}
    for b in range(B):
        k = kept[b].reshape(-1)
        cx = coords[b, ..., 0].reshape(-1)
        cy = coords[b, ..., 1].reshape(-1)
        pts = np.flatnonzero(k)
        lin = cx[pts].astype(np.int64) * NY + cy[pts]
        cnt = np.bincount(lin, minlength=NX * NY)
        w_all = (1.0 / np.maximum(cnt, 1)).astype(np.float32)
        order = np.lexsort((pts, cx[pts]))
        sp = pts[order]
        sx = cx[pts][order]
        sy = cy[pts][order]
        sw = w_all[lin[order]]
        new = np.ones(sp.size, bool)
        new[1:] = (np.diff(sx) != 0) | (np.diff(sp) > Lmax)
        starts = np.flatnonzero(new)
        ends = np.append(starts[1:], sp.size)
        for s, e in zip(starts, ends):
            key = (b, int(sx[s]))
            if key not in rows:
                rows[key] = {L: [] for L in CLASSES}
            i = s
            while i < e:
                j = i
                base = int(sp[i])
                while j < e and int(sp[j]) - base < Lmax:
                    j += 1
                span = int(sp[j - 1]) - base + 1
                L = min(c for c in CLASSES if c >= span)
                gstart = b * PB + base
                gstart = min(gstart, 2 * PB - L)
                off = b * PB + base - gstart
                ys = np.full(L, -1.0, np.float32)
                ws = np.zeros(L, np.float32)
                for t in range(i, j):
                    ys[int(sp[t]) - base + off] = float(sy[t])
                    ws[int(sp[t]) - base + off] = float(sw[t])
                rows[key][L].append((gstart, ys, ws))
                i = j
    return rows


def _assign_slots(rows, n_cores=8):
    keys = sorted(rows, key=lambda k: -sum(len(v) for v in rows[k].values()))
    cores = [[] for _ in range(n_cores)]
    load = [0] * n_cores
    for k in keys:
        cost = sum(len(v) for v in rows[k].values())
        ci = min(range(n_cores), key=lambda i: load[i])
        cores[ci].append(k)
        load[ci] += cost
    return cores, load


def _ceil(a, b):
    return -(-a // b)


def build_schedule(camera2lidar_rots, camera2lidar_trans):
    coords, kept = _geometry(camera2lidar_rots, camera2lidar_trans)
    rows = _build_runs(coords, kept)
    cores, load = _assign_slots(rows)
    n_cores = len(cores)
    NSLOTS = max(len(c) for c in cores)

    nchunks = {}
    for cls in CLASSES:
        Q = QUANT[cls]
        for w in range(NSLOTS):
            m = 0
            for ci in range(n_cores):
                if w < len(cores[ci]):
                    m = max(m, _ceil(len(rows[cores[ci][w]][cls]), Q))
            nchunks[(cls, w)] = m

    instrs = []
    for cls in CLASSES:
        Q = QUANT[cls]
        npc = P // Q
        stream = []
        for w in range(NSLOTS):
            stream += [(w, q) for q in range(nchunks[(cls, w)])]
        for i0 in range(0, len(stream), npc):
            instrs.append(dict(cls=cls, chunks=stream[i0 : i0 + npc]))
    instrs.sort(key=lambda r: (min(c[0] for c in r["chunks"]),
                               r["cls"], r["chunks"][0][1]))
    NINSTR = len(instrs)

    # per-instruction chunk content per core
    content = []
    for rec in instrs:
        cls = rec["cls"]
        Q = QUANT[cls]
        per_chunk = []
        for (w, q) in rec["chunks"]:
            cores_dat = []
            for ci in range(n_cores):
                dlist = rows[cores[ci][w]][cls] if w < len(cores[ci]) else []
                chunk = dlist[q * Q : (q + 1) * Q]
                starts = np.zeros(Q, np.int64)
                ys = np.full((Q, cls), -1.0, np.float32)
                ws = np.zeros((Q, cls), np.float32)
                for j, (st, yy, ww) in enumerate(chunk):
                    starts[j] = st
                    ys[j] = yy
                    ws[j] = ww
                cores_dat.append((starts, ys, ws))
            per_chunk.append(cores_dat)
        content.append(per_chunk)

    # segments (uniform): (lane, k0, k1, w, col) — emitted if any core has
    # a real point; col indexes the vid/invpc tables
    NMBUILD = 0
    slot_last = {}
    slot_first = {}
    for ii, rec in enumerate(instrs):
        cls = rec["cls"]
        Q = QUANT[cls]
        segs = []
        for lane in range(cls):
            groups = []
            for k, (w, q) in enumerate(rec["chunks"]):
                if groups and groups[-1][0] == w and groups[-1][2] == k:
                    groups[-1][2] = k + 1
                else:
                    groups.append([w, k, k + 1])
            for (w, k0, k1) in groups:
                occ = any(
                    (content[ii][k][ci][1][:, lane] >= 0).any()
                    for k in range(k0, k1) for ci in range(n_cores)
                )
                if occ:
                    segs.append([lane, k0 * Q, k1 * Q, w, NMBUILD])
                    NMBUILD += 1
                    if w not in slot_first:
                        slot_first[w] = (ii, len(segs) - 1)
                    slot_last[w] = (ii, len(segs) - 1)
        rec["segments"] = segs

    # start/stop flags per segment; copies/flushes per instruction
    for ii, rec in enumerate(instrs):
        rec["copies_after"] = []
        rec["flushes"] = []
        for si, seg in enumerate(rec["segments"]):
            w = seg[3]
            seg.append(slot_first[w] == (ii, si))
            seg.append(slot_last[w] == (ii, si))
    for w, (ii, si) in slot_last.items():
        instrs[ii]["copies_after"].append(w)
    nblocks = _ceil(NSLOTS, FLUSH_WINDOWS)
    for blk in range(nblocks):
        ws = [w for w in range(blk * FLUSH_WINDOWS,
                               min((blk + 1) * FLUSH_WINDOWS, NSLOTS))
              if w in slot_last]
        pos = max(slot_last[w][0] for w in ws) if ws else 0
        instrs[pos]["flushes"].append(blk)

    # per-core tables
    per_core = []
    for ci in range(n_cores):
        desc = np.zeros((P, NINSTR), np.int32)
        vid = np.full((P, max(NMBUILD, 1)), -1.0, np.float32)
        invpc = np.zeros((P, max(NMBUILD, 1)), np.float32)
        for ii, rec in enumerate(instrs):
            cls = rec["cls"]
            Q = QUANT[cls]
            for k in range(len(rec["chunks"])):
                starts, ys, ws = content[ii][k][ci]
                desc[k * Q : (k + 1) * Q, ii] = starts
            for seg in rec["segments"]:
                lane, lo, hi, w, col = seg[:5]
                for k in range(lo // Q, hi // Q):
                    starts, ys, ws = content[ii][k][ci]
                    vid[k * Q : (k + 1) * Q, col] = ys[:, lane]
                    invpc[k * Q : (k + 1) * Q, col] = ws[:, lane]
        slot_rows = [cores[ci][w] if w < len(cores[ci]) else None
                     for w in range(NSLOTS)]
        per_core.append(dict(desc=desc, vid=vid, invpc=invpc,
                             slot_rows=slot_rows))

    return dict(instrs=instrs, NINSTR=NINSTR, NMBUILD=NMBUILD,
                NSLOTS=NSLOTS, per_core=per_core, load=load,
                nblocks=nblocks)


# ---------------------------------------------------------------- device


def build_program(sched):
    import concourse.bacc as bacc
    import concourse.bass as bass
    import concourse.mybir as mybir
    import concourse.tile as tile

    f32, f16 = mybir.dt.float32, mybir.dt.float16
    i32 = mybir.dt.int32
    NINSTR, NMBUILD = sched["NINSTR"], sched["NMBUILD"]
    NSLOTS = sched["NSLOTS"]

    nc = bacc.Bacc(None)
    xb = nc.declare_dram_parameter("xb", [2 * PB, C], f16, isOutput=False)
    desc_d = nc.declare_dram_parameter("desc", [P, NINSTR], i32,
                                       isOutput=False)
    vid_d = nc.declare_dram_parameter("vid", [P, NMBUILD], f32,
                                      isOutput=False)
    invpc_d = nc.declare_dram_parameter("invpc", [P, NMBUILD], f32,
                                        isOutput=False)
    iota_d = nc.declare_dram_parameter("iota", [P, NY], f16, isOutput=False)
    out_d = nc.declare_dram_parameter("out", [C, NSLOTS * NY], f32,
                                      isOutput=True)

    with tile.TileContext(nc) as tc:
        with (
            tc.tile_pool(name="const", bufs=1) as cpool,
            tc.tile_pool(name="g2", bufs=12) as g2pool,
            tc.tile_pool(name="m", bufs=8) as mpool,
            tc.tile_pool(name="psum", bufs=6, space="PSUM") as ppool,
            tc.tile_pool(name="hotp", bufs=1, space="PSUM") as hpool,
            tc.tile_pool(name="slab", bufs=3) as slabpool,
        ):
            desc_t = cpool.tile([P, NINSTR], i32)
            vid_t = cpool.tile([P, NMBUILD], f32)
            invpc_t = cpool.tile([P, NMBUILD], f32)
            iota_t = cpool.tile([P, NY], f16)
            nc.sync.dma_start(out=desc_t[:], in_=desc_d[:])
            nc.sync.dma_start(out=vid_t[:], in_=vid_d[:])
            nc.sync.dma_start(out=invpc_t[:], in_=invpc_d[:])
            nc.sync.dma_start(out=iota_t[:], in_=iota_d[:])

            # scratch PSUM tile kept warm with one tiny matmul per gather:
            # prevents the PE HAM clock-gate from re-throttling to 1.2 GHz
            # during gather-bound stretches (real matmuls then run ~2x).
            hot = hpool.tile([C, NY], f32, tag="hot", name="hotplate")

            wtiles = {}
            slabs = {}
            PAIR = 1  # one gather per instruction (multi-offset unsupported)
            L = CLASSES[0]
            gtiles = {}
            for ii, rec in enumerate(sched["instrs"]):
                if ii % PAIR == 0:
                    npair = min(PAIR, NINSTR - ii)
                    gw = g2pool.tile([P, npair * L * C], f16, tag="g")
                    nc.gpsimd.indirect_dma_start(
                        out=gw[:],
                        out_offset=None,
                        in_=xb[:],
                        in_offset=bass.IndirectOffsetOnAxis(
                            ap=desc_t[:, ii : ii + npair], axis=0
                        ),
                    )
                    for j in range(npair):
                        gtiles[ii + j] = (gw, j * L * C)
                    nc.tensor.matmul(
                        hot[:], iota_t[:, :C], iota_t[:],
                        start=True, stop=True, skip_group_check=True,
                    )
                g, goff = gtiles.pop(ii)
                for seg in rec["segments"]:
                    lane, lo, hi, w, col, st, sp_ = seg
                    M = mpool.tile([P, NY], f16, tag="m")
                    nc.vector.tensor_scalar(
                        out=M[:],
                        in0=iota_t[:],
                        scalar1=vid_t[:, col : col + 1],
                        scalar2=invpc_t[:, col : col + 1],
                        op0=mybir.AluOpType.is_equal,
                        op1=mybir.AluOpType.mult,
                    )
                    if st:
                        wtiles[w] = ppool.tile([C, NY], f32, tag="w",
                                               name=f"w{w}")
                    nc.tensor.matmul(
                        wtiles[w][:],
                        g[:, goff + lane * C : goff + (lane + 1) * C],
                        M[:],
                        start=st,
                        stop=sp_,
                        skip_group_check=True,
                    )
                for w in rec["copies_after"]:
                    blk = w // FLUSH_WINDOWS
                    if blk not in slabs:
                        slabs[blk] = slabpool.tile(
                            [C, FLUSH_WINDOWS * NY], f32, tag="slab",
                            name=f"slab{blk}",
                        )
                    off = w % FLUSH_WINDOWS
                    nc.scalar.activation(
                        out=slabs[blk][:, off * NY : (off + 1) * NY],
                        in_=wtiles.pop(w)[:],
                        func=mybir.ActivationFunctionType.Copy,
                    )
                for blk in rec["flushes"]:
                    w0 = blk * FLUSH_WINDOWS
                    w1 = min(w0 + FLUSH_WINDOWS, NSLOTS)
                    nc.sync.dma_start(
                        out=out_d[:, w0 * NY : w1 * NY],
                        in_=slabs.pop(blk)[:, : (w1 - w0) * NY],
                    )
    nc.compile()
    return nc


def make_in_maps(sched, x):
    xcat = np.ascontiguousarray(
        x.reshape(2 * PB, C)).astype(np.float16)
    iota = np.broadcast_to(
        np.arange(NY, dtype=np.float16)[None, :], (P, NY)
    ).copy()
    in_maps = []
    for ci in range(8):
        pc = sched["per_core"][ci]
        in_maps.append(
            {
                "xb": xcat,
                "desc": pc["desc"],
                "vid": pc["vid"],
                "invpc": pc["invpc"],
                "iota": iota,
            }
        )
    return in_maps


def assemble(slabs, sched):
    out = np.zeros((B, C, NX, NY), np.float32)
    for ci in range(8):
        pc = sched["per_core"][ci]
        slab = slabs[ci]
        for s, key in enumerate(pc["slot_rows"]):
            if key is None:
                continue
            b, xrow = key
            out[b, :, xrow, :] = slab[:, s * NY : (s + 1) * NY]
    return out


def kernel(x, camera2lidar_rots, camera2lidar_trans):
    from concourse.bass_utils import run_bass_kernel_spmd

    x = np.asarray(x, dtype=np.float32)
    rots = np.asarray(camera2lidar_rots, dtype=np.float32)
    trans = np.asarray(camera2lidar_trans, dtype=np.float32)
    sched = build_schedule(rots, trans)
    nc = build_program(sched)
    in_maps = make_in_maps(sched, x)
    res = run_bass_kernel_spmd(nc, in_maps, list(range(8)))
    slabs = [res.results[ci]["out"] for ci in range(8)]
    return assemble(slabs, sched)
